# revision 1
# baseline (speedup 1.0000x reference)
"""Trainium2 Bass kernel for nn_CompressedSparseLayerELSA.

Computes out = relu(x @ Am @ Am.T - x) where
  Am = row_normalize(top64_by_abs_mask(A)),  x:[1024,50000] f32, A:[50000,256] f32.

Sharding: items (50000) split 8 ways. Each core gets x[:, shard] and A[shard, :].

Per core:
  Phase 1 (per 128-item chunk): load A chunk; exact top-64 mask via 8 rounds
    of DVE max8 + match_replace (match_replace marks the selected positions
    with -1, reproducing jax.lax.top_k's lowest-index tie-break exactly);
    mask+sign-restore on the Pool engine; sum-of-squares via ACT Square
    accumulation; normalize; transpose Am halves on the PE (fp32r bitcast);
    load + PE-transpose the x chunk, storing xT in SBUF as bf16 (resident for
    the whole kernel); accumulate xA^T [256,1024] in PSUM with bf16 matmuls.
  Phase 2: all-reduce xA^T across the 8 cores (1MB via DRAM staging).
  Phase 3: out[:, shard] = relu(xA @ AmT - x) with the -x folded into the
    PSUM accumulation via (-I) matmuls whose stationary operands are the
    resident bf16 xT chunks -- no second HBM read of x.
"""

import sys

sys.path.insert(0, "/opt/trn_rl_repo")

import numpy as np

import concourse.bacc as bacc
import concourse.mybir as mybir
import concourse.tile as tile
from concourse.bass_utils import run_bass_kernel_spmd
from concourse.masks import make_identity

dt = mybir.dt
AF = mybir.ActivationFunctionType
OP = mybir.AluOpType

N_CORES = 8
D = 256            # n_dims
K = 64             # top-k kept per row

_cache = {}


def _ceil_div(a, b):
    return (a + b - 1) // b


def build(B, I_SHARD, n_cores):
    """Build the SPMD Bacc program for one core's shard."""
    nc = bacc.Bacc("TRN2", target_bir_lowering=False, debug=False,
                   num_devices=n_cores)

    x_in = nc.dram_tensor("x_in", [B, I_SHARD], dt.float32, kind="ExternalInput").ap()
    a_in = nc.dram_tensor("a_in", [I_SHARD, D], dt.float32, kind="ExternalInput").ap()
    o_out = nc.dram_tensor("o_out", [B, I_SHARD], dt.float32, kind="ExternalOutput").ap()

    NB = B // 128                     # batch chunks of 128
    NC = _ceil_div(I_SHARD, 128)      # item chunks of 128 (last may be ragged)
    NJ = _ceil_div(I_SHARD, 512)      # item blocks of 512 for phase-3 output
    last_c = I_SHARD - (NC - 1) * 128     # rows in last item chunk
    NH = B // 512                     # 512-wide batch halves for transposes

    with tile.TileContext(nc) as tc:
        with (
            tc.tile_pool(name="const", bufs=1) as const_pool,
            tc.tile_pool(name="res", bufs=1) as res_pool,
            tc.tile_pool(name="dram", bufs=1, space="DRAM") as dram_pool,
        ):
            # ---- constants
            ident = const_pool.tile([128, 128], dt.float32)
            make_identity(nc, ident)
            ident_r = const_pool.tile([128, 128], dt.float32r)
            nc.scalar.copy(out=ident_r, in_=ident)
            neg_ident = const_pool.tile([128, 128], dt.float32)
            nc.gpsimd.memset(neg_ident, 0.0)
            nc.gpsimd.affine_select(
                out=neg_ident, in_=neg_ident, compare_op=OP.not_equal,
                fill=-1.0, base=0, pattern=[[-1, 128]], channel_multiplier=1)
            neg_ident_b = const_pool.tile([128, 128], dt.bfloat16)
            nc.scalar.copy(out=neg_ident_b, in_=neg_ident)

            # ---- residents: xT (items-major x, bf16) and AmT halves (bf16)
            xt_all = res_pool.tile([128, NC * B], dt.bfloat16)
            amt = [res_pool.tile([128, I_SHARD], dt.bfloat16, name=f"amt{d}")
                   for d in range(2)]
            xat_b = [res_pool.tile([128, B], dt.bfloat16, name=f"xatb{d}")
                     for d in range(2)]

            with (
                tc.tile_pool(name="a_io", bufs=3) as a_pool,
                tc.tile_pool(name="tk", bufs=3) as tk_pool,
                tc.tile_pool(name="tk8", bufs=3) as tk8_pool,
                tc.tile_pool(name="x_io", bufs=3) as x_pool,
                tc.tile_pool(name="ps_t", bufs=2, space="PSUM") as ps_t_pool,
                tc.tile_pool(name="ps_acc", bufs=1, space="PSUM") as ps_acc_pool,
            ):
                # phase 1 accumulators: xA^T [256, B] f32 in PSUM (2 banks ea)
                ps_xat = [ps_acc_pool.tile([128, B], dt.float32,
                                           name=f"psxat{d}")
                          for d in range(2)]

                # ==== phase 1: topk -> Am -> AmT; x -> xT; mm1, per item-chunk
                for c in range(NC):
                    rows = 128 if c < NC - 1 else last_c
                    i0 = c * 128

                    # --- load A chunk
                    a_t = a_pool.tile([128, D], dt.float32, name="a_t")
                    nc.sync.dma_start(out=a_t[:rows], in_=a_in[i0:i0 + rows])
                    absa = tk_pool.tile([128, D], dt.float32, name="absa")
                    nc.scalar.activation(absa[:rows], a_t[:rows], AF.Abs)

                    # --- exact top-64 marking: 8 rounds of max8+match_replace
                    # (first-occurrence replacement == lax.top_k tie-break)
                    wrk = tk_pool.tile([128, D], dt.float32, name="wrk")
                    m8 = tk8_pool.tile([128, 8], dt.float32, name="m8")
                    src = absa
                    for r in range(K // 8):
                        nc.vector.max(out=m8[:rows], in_=src[:rows])
                        nc.vector.match_replace(
                            out=wrk[:rows], in_to_replace=m8[:rows],
                            in_values=src[:rows], imm_value=-1.0)
                        src = wrk

                    # --- mask (selected iff wrk<0), restore sign
                    az = tk_pool.tile([128, D], dt.float32, name="az")
                    nc.vector.scalar_tensor_tensor(
                        out=az[:rows], in0=wrk[:rows], scalar=0.0,
                        in1=a_t[:rows], op0=OP.is_lt, op1=OP.mult)

                    # --- sum of squares on ACT, then rn = 1/sqrt(ss)
                    sqd = tk_pool.tile([128, D], dt.float32, name="sqd")
                    ss = tk8_pool.tile([128, 1], dt.float32, name="ss")
                    nc.scalar.activation(sqd[:rows], az[:rows], AF.Square,
                                         accum_out=ss[:rows])
                    rn = tk8_pool.tile([128, 1], dt.float32, name="rn")
                    nc.scalar.activation(rn[:rows], ss[:rows], AF.Sqrt)
                    nc.vector.reciprocal(rn[:rows], rn[:rows])

                    # --- normalized Am: fp32r (for PE transpose) + bf16 (mm1)
                    am = tk_pool.tile([128, D], dt.float32r, name="am")
                    nc.scalar.activation(am[:rows], az[:rows], AF.Copy,
                                         scale=rn[:rows])
                    am_b = tk_pool.tile([128, D], dt.bfloat16, name="am_b")
                    nc.scalar.activation(am_b[:rows], az[:rows], AF.Copy,
                                         scale=rn[:rows])

                    # --- AmT via PE transpose (fp32r), PSUM -> bf16 SBUF
                    for d in range(2):
                        pst = ps_t_pool.tile([128, 512], dt.float32r,
                                             name="pst")
                        nc.tensor.transpose(
                            pst[:, :rows],
                            am[:rows, d * 128:(d + 1) * 128],
                            ident_r[:rows, :rows])
                        nc.scalar.copy(out=amt[d][:, i0:i0 + rows],
                                       in_=pst[:, :rows])

                    # --- x chunk -> resident bf16 xT (PE transpose per half)
                    for h in range(NH):
                        b0 = h * 512
                        xb3 = x_pool.tile([128, 4, 128], dt.float32,
                                          name="xb3")
                        nc.sync.dma_start(
                            out=xb3[:, :, :rows],
                            in_=x_in[b0:b0 + 512, i0:i0 + rows].rearrange(
                                "(q p) i -> p q i", p=128))
                        pst2 = ps_t_pool.tile([128, 512], dt.float32,
                                              name="pst2")
                        for q in range(4):
                            nc.tensor.transpose(
                                pst2[:rows, q * 128:q * 128 + 128],
                                xb3[:, q, :rows],
                                ident)
                        nc.scalar.copy(
                            out=xt_all[:rows, c * B + b0:c * B + b0 + 512],
                            in_=pst2[:rows])

                    # --- mm1: xA^T += Am_chunk^T-part (bf16, 512-wide halves)
                    for d in range(2):
                        for h in range(NH):
                            nc.tensor.matmul(
                                ps_xat[d][:, h * 512:(h + 1) * 512],
                                am_b[:rows, d * 128:(d + 1) * 128],
                                xt_all[:rows, c * B + h * 512:
                                       c * B + (h + 1) * 512],
                                start=(c == 0), stop=(c == NC - 1))

                # ==== phase 2a: xA^T PSUM -> SBUF -> DRAM
                cc_in = dram_pool.tile([2 * 128, B], dt.float32)
                for d in range(2):
                    xat_sb = res_pool.tile([128, B], dt.float32,
                                           name=f"xat_sb{d}")
                    nc.scalar.copy(out=xat_sb, in_=ps_xat[d])
                    nc.sync.dma_start(out=cc_in[d * 128:(d + 1) * 128],
                                      in_=xat_sb)

            # ==== phase 2b: all-reduce across cores, round to bf16
            cc_out = dram_pool.tile([2 * 128, B], dt.float32,
                                    addr_space="Shared")
            nc.gpsimd.collective_compute(
                "AllReduce", OP.add,
                replica_groups=[list(range(n_cores))],
                ins=[cc_in.opt()], outs=[cc_out.opt()])
            for d in range(2):
                xat_f = res_pool.tile([128, B], dt.float32, name=f"xat_f{d}")
                nc.sync.dma_start(out=xat_f, in_=cc_out[d * 128:(d + 1) * 128])
                nc.scalar.copy(out=xat_b[d], in_=xat_f)

            # ==== phase 3: out[:, shard] = relu(xA @ AmT - x)
            with (
                tc.tile_pool(name="ep", bufs=4) as ep_pool,
                tc.tile_pool(name="ps_o", bufs=4, space="PSUM") as ps_o_pool,
            ):
                for b in range(NB):
                    for j in range(NJ):
                        w = 512 if (j < NJ - 1 or I_SHARD % 512 == 0) \
                            else I_SHARD % 512
                        j0 = j * 512
                        ps_o = ps_o_pool.tile([128, 512], dt.float32,
                                              name="ps_o")
                        # xA @ AmT accumulation first (start=True is a
                        # BANK-level has_written clear, so it must be the
                        # single first matmul on this bank)
                        for d in range(2):
                            nc.tensor.matmul(
                                ps_o[:, :w],
                                xat_b[d][:, b * 128:(b + 1) * 128],
                                amt[d][:, j0:j0 + w],
                                start=(d == 0), stop=False)
                        # -x fold: per 128-item block, stationary = resident
                        # bf16 xT chunk, moving = -I, accumulating into ps_o
                        nq = _ceil_div(w, 128)
                        for q in range(nq):
                            c3 = j * 4 + q
                            rr = min(128, w - q * 128)
                            col = c3 * B + b * 128
                            nc.tensor.matmul(
                                ps_o[:, q * 128:q * 128 + rr],
                                xt_all[:rr, col:col + 128],
                                neg_ident_b[:rr, :rr],
                                start=False, stop=(q == nq - 1))
                        o_sb = ep_pool.tile([128, 512], dt.float32,
                                            name="o_sb")
                        nc.scalar.activation(o_sb[:, :w], ps_o[:, :w], AF.Relu)
                        nc.sync.dma_start(
                            out=o_out[b * 128:(b + 1) * 128, j0:j0 + w],
                            in_=o_sb[:, :w])

    nc.compile()
    return nc


def _get_program(B, I_SHARD, n_cores):
    key = (B, I_SHARD, n_cores)
    if key not in _cache:
        _cache[key] = build(B, I_SHARD, n_cores)
    return _cache[key]


last_exec_time_ns = None
last_result = None


def kernel(x: np.ndarray, A: np.ndarray) -> np.ndarray:
    global last_exec_time_ns, last_result
    x = np.asarray(x)
    A = np.asarray(A)
    B, I = x.shape
    assert A.shape == (I, D), (A.shape, I)
    i_shard = I // N_CORES
    nc = _get_program(B, i_shard, N_CORES)
    in_maps = [
        {
            "x_in": np.ascontiguousarray(x[:, c * i_shard:(c + 1) * i_shard]),
            "a_in": np.ascontiguousarray(A[c * i_shard:(c + 1) * i_shard]),
        }
        for c in range(N_CORES)
    ]
    res = run_bass_kernel_spmd(nc, in_maps, list(range(N_CORES)))
    last_exec_time_ns = res.exec_time_ns
    last_result = res
    out = np.concatenate([res.results[c]["o_out"] for c in range(N_CORES)],
                         axis=1)
    return out.astype(np.float32, copy=False)

